# revision 31
# baseline (speedup 1.0000x reference)
"""Causal self-attention with rotary embeddings (B=2, T=2048, D=1024, H=16,
d_k=64) on 8 Trainium2 NeuronCores.

Sharding: core c handles batch b = c//4 and 4 heads (c%4)*4..+4 — data
parallel on B, tensor parallel on heads.  Each core computes its heads'
qkv projection, RoPE, causal attention, and a partial output projection
over its 256 attention channels; the host sums the 4 partials per batch.

Layout tricks (v2, all-bf16):
  * every matmul operand is bf16 (1 cycle/row at any moving size, vs
    f32r's 4x penalty under 256) — end-to-end rel err ~5e-3 vs the 2e-2
    budget.
  * q/k channels are de-interleaved host-side (RoPE pair -> half-split
    form) packed 2 heads per 128-partition tile.
  * the RoPE +/-32-partition swap is a pure-permutation matmul on
    TensorE (SBUF->SBUF DMA hard-faults the device); the sin-table sign
    is folded per-partition host-side so q and k share one evict path.
  * scores are computed unscaled; the 1/sqrt(d_k) factor rides the exp
    activation's free scale field.
  * causal diagonal blocks slice the matmul moving dim to [ls:512] for
    scores AND attn@v (the e tile's masked-out region is never touched,
    so the gpsimd memsets are gone too).
  * softmax skips max-subtraction; the denominator is an extra
    ones-column of v folded into attn@v; divide is a per-head
    broadcast-reciprocal multiply at eviction.
  * biases are all-zero in this problem; a with_bias program variant
    (ones-row augmentation, like v1) is compiled only if a nonzero bias
    ever shows up.
"""

import sys

sys.path.insert(0, "/opt/trn_rl_repo")

import numpy as np
import ml_dtypes

import concourse.bacc as bacc
import concourse.tile as tile
from concourse import mybir
from concourse.bass_utils import run_bass_kernel_spmd

F32 = mybir.dt.float32
BF16 = mybir.dt.bfloat16

B, T, D = 2, 2048, 1024
NH, DK = 16, 64
THETA = 10000.0
NCORES = 8
HEADS_PER_CORE = 4

TC512 = T // 512        # 4   i-chunks of 512
TC128 = T // 128        # 16  t/j-chunks of 128
KC = D // 128           # 8   d_model contraction chunks


def build_program(debug=False, with_bias=False):
    nc = bacc.Bacc("TRN2", target_bir_lowering=False, debug=False)

    # all inputs arrive pre-packed host-side as the exact SBUF image
    # ([128 partitions, free]) so every DMA is one descriptor with large
    # per-partition-contiguous packets (the per-queue DMA rate is packet-
    # bound: ~17ns per packet, so 1KB packets cap a queue at ~58GB/s)
    XT = nc.dram_tensor("XT", [128, KC * T], BF16, kind="ExternalInput").ap()
    WQK = nc.dram_tensor("WQK", [128, KC * 512], BF16, kind="ExternalInput").ap()
    WV = nc.dram_tensor("WV", [128, KC * 256], BF16, kind="ExternalInput").ap()
    WOUT = nc.dram_tensor("WOUT", [128, 2 * D], BF16, kind="ExternalInput").ap()
    PSW = nc.dram_tensor("PSW", [128, 128], BF16, kind="ExternalInput").ap()
    CQ = nc.dram_tensor("CQ", [128, T], BF16, kind="ExternalInput").ap()
    SQ = nc.dram_tensor("SQ", [128, T], BF16, kind="ExternalInput").ap()
    TRI = nc.dram_tensor("TRI", [128, 128], BF16, kind="ExternalInput").ap()
    OUT = nc.dram_tensor("OUT", [128, TC128 * D], BF16, kind="ExternalOutput").ap()
    if with_bias:
        XLAST = nc.dram_tensor("XLAST", [1, T], BF16, kind="ExternalInput").ap()
        WQKLAST = nc.dram_tensor("WQKLAST", [1, 512], BF16, kind="ExternalInput").ap()
        WVLAST = nc.dram_tensor("WVLAST", [1, 256], BF16, kind="ExternalInput").ap()
    if debug:
        DBG_QKT = nc.dram_tensor("DBG_QKT", [128, 4 * T], BF16, kind="ExternalOutput").ap()
        DBG_V = nc.dram_tensor("DBG_V", [128, TC128 * 260], BF16, kind="ExternalOutput").ap()
        DBG_ATT = nc.dram_tensor("DBG_ATT", [128, 2 * T], BF16, kind="ExternalOutput").ap()

    EXP = mybir.ActivationFunctionType.Exp

    with tile.TileContext(nc) as tc:
        with (
            tc.tile_pool(name="persist", bufs=1) as persist,
            tc.tile_pool(name="p1w", bufs=1) as p1w,
            tc.tile_pool(name="p1t", bufs=3) as p1t,
            tc.tile_pool(name="p2e", bufs=4) as p2e,
            tc.tile_pool(name="p2bc", bufs=1) as p2bc,
            tc.tile_pool(name="p2r", bufs=1) as p2r,
            tc.tile_pool(name="pj", bufs=2, space="PSUM") as pj,
            tc.tile_pool(name="sps", bufs=2, space="PSUM") as sps,
            tc.tile_pool(name="avps", bufs=2, space="PSUM") as avps,
        ):
            # ---- persistent tiles --------------------------------------
            qkT = persist.tile([128, 4 * T], BF16, tag="qkT")       # Qp0 Kp0 Qp1 Kp1
            v_sb = persist.tile([128, TC128 * 260], BF16, tag="v_sb")  # [jc, head, 64+1]
            attnT = persist.tile([128, 2 * T], BF16, tag="attnT")   # c-chunks x t
            wout_sb = persist.tile([128, 2 * D], BF16, tag="wout_sb")
            tri_sb = persist.tile([128, 128], BF16, tag="tri_sb")

            x_sb = p1w.tile([128, KC * T], BF16, tag="x_sb")
            wqk_sb = p1w.tile([128, KC * 512], BF16, tag="wqk_sb")
            wv_sb = p1w.tile([128, KC * 256], BF16, tag="wv_sb")
            cq_sb = p1w.tile([128, T], BF16, tag="cq_sb")
            sq_sb = p1w.tile([128, T], BF16, tag="sq_sb")
            psw_sb = p1w.tile([128, 128], BF16, tag="psw_sb")
            if with_bias:
                xlast = p1w.tile([1, T], BF16, tag="xlast")
                wqk_last = p1w.tile([1, 512], BF16, tag="wqk_last")
                wv_last = p1w.tile([1, 256], BF16, tag="wv_last")

            # x layout (SBUF and DRAM image): cols n*4096 + k*512 + c, so a
            # t-block load is one contiguous 8KB-per-partition descriptor
            def xsl(n, k, lo=0, hi=512):
                base = n * (KC * 512) + k * 512
                return x_sb[:, base + lo:base + hi]

            def load_x_block(n, eng=None):
                (eng or nc.sync).dma_start(
                    x_sb[:, n * 4096:(n + 1) * 4096], XT[:, n * 4096:(n + 1) * 4096])

            # split the critical first-wave loads into halves so the k=0
            # matmuls start as soon as the first half lands; defer the rest
            nc.gpsimd.dma_start(x_sb[:, 0:2048], XT[:, 0:2048])
            nc.sync.dma_start(wqk_sb[:, 0:2048], WQK[:, 0:2048])
            nc.gpsimd.dma_start(x_sb[:, 2048:4096], XT[:, 2048:4096])
            nc.sync.dma_start(wqk_sb[:, 2048:4096], WQK[:, 2048:4096])
            nc.scalar.dma_start(cq_sb[:], CQ[:])
            nc.scalar.dma_start(sq_sb[:], SQ[:])
            nc.gpsimd.dma_start(wv_sb[:], WV[:])
            nc.scalar.dma_start(psw_sb[:], PSW[:])
            nc.scalar.dma_start(tri_sb[:], TRI[:])
            if with_bias:
                nc.scalar.dma_start(wqk_last[:], WQKLAST[:])
                nc.scalar.dma_start(xlast[:], XLAST[:])
                nc.scalar.dma_start(wv_last[:], WVLAST[:])

            # ones columns of v_aug: one strided memset
            v4 = v_sb[:].rearrange("p (jc h e) -> p jc h e", jc=TC128, h=4)
            nc.vector.memset(v4[:, :, :, 64:65], 1.0)

            # ---------------- building blocks ---------------------------
            def qk_proj_chunk(m, n):
                """project q/k m-chunk (128 channels) for t-chunk n (512);
                rope: r = ps*cos + swap32(ps*sin±) with the swap as two
                SBUF->SBUF DMAs."""
                nsl = slice(n * 512, (n + 1) * 512)
                ps = pj.tile([128, 512], F32, tag="pj", name=f"psqk_{m}_{n}")
                for k in range(KC):
                    nc.tensor.matmul(
                        ps[:],
                        wqk_sb[:, k * 512 + m * 128:k * 512 + (m + 1) * 128],
                        xsl(n, k),
                        start=(k == 0), stop=(not with_bias and k == KC - 1),
                    )
                if with_bias:
                    nc.tensor.matmul(
                        ps[:], wqk_last[:, m * 128:(m + 1) * 128], xlast[:, nsl],
                        start=False, stop=True,
                    )
                tmp_s = p1t.tile([128, 512], BF16, tag="tmp_s", name=f"tmps_{m}_{n}")
                tmp_c = p1t.tile([128, 512], BF16, tag="tmp_c", name=f"tmpc_{m}_{n}")
                nc.vector.tensor_mul(tmp_s[:], ps[:], sq_sb[:, nsl])
                nc.vector.tensor_mul(tmp_c[:], ps[:], cq_sb[:, nsl])
                sw = pj.tile([128, 512], F32, tag="pj", name=f"sw_{m}_{n}")
                nc.tensor.matmul(sw[:], psw_sb[:], tmp_s[:], start=True, stop=True)
                nc.vector.tensor_add(qkT[:, m * T + n * 512:m * T + (n + 1) * 512], sw[:], tmp_c[:])

            def v_proj_chunk(tcc):
                tsl = slice(tcc * 128, (tcc + 1) * 128)
                psv = pj.tile([128, 256], F32, tag="pj", name=f"psv_{tcc}")
                for k in range(KC):
                    nc.tensor.matmul(
                        psv[:],
                        xsl(tcc // 4, k, (tcc % 4) * 128, (tcc % 4 + 1) * 128),
                        wv_sb[:, k * 256:(k + 1) * 256],
                        start=(k == 0), stop=(not with_bias and k == KC - 1),
                    )
                if with_bias:
                    nc.tensor.matmul(psv[:], xlast[:, tsl], wv_last[:], start=False, stop=True)
                vdst = v_sb[:, tcc * 260:(tcc + 1) * 260].rearrange(
                    "p (h e) -> p h e", h=4)[:, :, 0:64]
                vsrc = psv[:].rearrange("p (h e) -> p h e", e=64)
                nc.vector.tensor_copy(vdst, vsrc)

            def attn_ic(p, ic, fillers=()):
                """attention for head-pair p, query chunk ic (512 queries).
                fillers: callables run one per jc iteration (PE density)."""
                fillers = list(fillers)
                qof = (2 * p) * T
                kof = (2 * p + 1) * T
                njc = 4 * ic + 4
                av = [avps.tile([65, 512], F32, tag="av", name=f"av_{p}_{ic}_{i}") for i in range(2)]
                for jc in range(njc):
                    rel = jc - 4 * ic
                    ls = 0 if rel < 0 else rel * 128
                    e_pair = p2e.tile([128, 1024], BF16, tag="e_t", name=f"e_{p}_{ic}_{jc}")
                    s_pair = sps.tile([128, 1024], F32, tag="s_ps", name=f"s_{p}_{ic}_{jc}")
                    for hh in range(2):
                        pof = hh * 64
                        nc.tensor.matmul(
                            s_pair[:, hh * 512 + ls:(hh + 1) * 512],
                            qkT[pof:pof + 64, kof + jc * 128:kof + (jc + 1) * 128],
                            qkT[pof:pof + 64, qof + ic * 512 + ls:qof + (ic + 1) * 512],
                            start=True, stop=True,
                        )
                    sv = s_pair[:].rearrange("p (h w) -> p h w", h=2)
                    ev = e_pair[:].rearrange("p (h w) -> p h w", h=2)
                    nc.scalar.activation(ev[:, :, ls:512], sv[:, :, ls:512], EXP, scale=0.125)
                    if rel >= 0:
                        tsl_ = slice(rel * 128, (rel + 1) * 128)
                        nc.gpsimd.tensor_mul(ev[:, :, tsl_], ev[:, :, tsl_],
                                             tri_sb[:].unsqueeze(1).broadcast_to([128, 2, 128]))
                    # fillers sit between the scores/exp and the dependent
                    # attn@v matmuls: the tensor queue chews them while exp
                    # (and, at jc==0, the previous block's av eviction)
                    # completes instead of head-of-line stalling
                    if fillers and (jc % max(1, njc // len(fillers)) == 0 or jc == njc - 1):
                        while fillers and len(fillers) > (njc - 1 - jc):
                            fillers.pop(0)()
                    for hh in range(2):
                        nc.tensor.matmul(
                            av[hh][:, ls:512],
                            v_sb[:, jc * 260 + (2 * p + hh) * 65:jc * 260 + (2 * p + hh) * 65 + 65],
                            e_pair[:, hh * 512 + ls:(hh + 1) * 512],
                            start=(jc == 0), stop=(jc == njc - 1),
                            skip_group_check=True,
                        )
                den, rec, bc = [], [], []
                for hh in range(2):
                    den.append(p2r.tile([1, 512], F32, tag=f"den{hh}", name=f"den_{p}_{ic}_{hh}"))
                    nc.vector.tensor_copy(den[hh][:], av[hh][64:65, :])
                for hh in range(2):
                    rec.append(p2r.tile([1, 512], F32, tag=f"rec{hh}", name=f"rec_{p}_{ic}_{hh}"))
                    nc.vector.reciprocal_approx_fast(rec[hh][:], den[hh][:])
                for hh in range(2):
                    bc.append(p2bc.tile([64, 512], F32, tag=f"bc{hh}", name=f"bc_{p}_{ic}_{hh}"))
                    nc.gpsimd.partition_broadcast(bc[hh][:], rec[hh][:], channels=64)
                for hh in range(2):
                    head = 2 * p + hh
                    cof = (head // 2) * T
                    pof = (head % 2) * 64
                    nc.vector.tensor_mul(
                        attnT[pof:pof + 64, cof + ic * 512:cof + (ic + 1) * 512],
                        av[hh][0:64, :], bc[hh][:],
                    )

            def out_proj_chunk(tcc):
                po_sb = p1t.tile([128, 1024], BF16, tag="po_sb", name=f"po_sb_{tcc}")
                for oc in range(2):
                    po = pj.tile([128, 512], F32, tag="pj", name=f"po_{tcc}_{oc}")
                    for cc in range(2):
                        nc.tensor.matmul(
                            po[:],
                            attnT[:, cc * T + tcc * 128:cc * T + (tcc + 1) * 128],
                            wout_sb[:, cc * D + oc * 512:cc * D + (oc + 1) * 512],
                            start=(cc == 0), stop=(cc == 1),
                        )
                    osl = slice(oc * 512, (oc + 1) * 512)
                    if oc == 0:
                        nc.vector.tensor_copy(po_sb[:, osl], po[:])
                    else:
                        nc.scalar.copy(po_sb[:, osl], po[:])
                (nc.sync if tcc % 2 == 0 else nc.scalar).dma_start(
                    OUT[:, tcc * D:(tcc + 1) * D], po_sb[:])

            # ---------------- schedule: n-major waves -------------------
            # wave n: project all qk m-chunks + v chunks for t-block n, run
            # both pairs' attention for query block n, and the out
            # projection for t-chunks completed in wave n-1.
            load_x_block(1)
            for m in range(4):
                qk_proj_chunk(m, 0)
            for tcc in range(4):
                v_proj_chunk(tcc)
            for n in range(TC512):
                fill0, fill1 = [], []
                if n < 3:
                    nx = n + 1
                    if nx + 1 < TC512:
                        fill0 += [lambda b=nx + 1: load_x_block(b)]
                    fill0 += [(lambda m=m: qk_proj_chunk(m, nx)) for m in range(4)]
                    fill0 += [(lambda t=t: v_proj_chunk(t)) for t in range(4 * nx, 4 * nx + 4)]
                if n == 0:
                    def load_wout():
                        nc.sync.dma_start(wout_sb[:], WOUT[:])
                    fill0 += [load_wout]
                # rebalance out-proj fillers toward the long final wave;
                # chunks 10-11 are held back to fill the final evict window
                op_sched = {1: range(0, 4), 2: range(4, 6), 3: range(6, 10)}
                if n in op_sched:
                    fill1 += [(lambda t=t: out_proj_chunk(t)) for t in op_sched[n]]
                half = len(fill0) // 2
                attn_ic(0, n, fill0[:half] + fill1[:2])
                attn_ic(1, n, fill0[half:] + fill1[2:])
            for tcc in range(10, 16):
                out_proj_chunk(tcc)

            if debug:
                nc.sync.dma_start(DBG_QKT[:], qkT[:])
                nc.sync.dma_start(DBG_V[:], v_sb[:])
                nc.sync.dma_start(DBG_ATT[:], attnT[:])

    nc.compile()
    return nc


_DEINT = list(range(0, DK, 2)) + list(range(1, DK, 2))


def _rope_tables():
    j = np.arange(DK // 2, dtype=np.float64)
    inv_freq = THETA ** (-2.0 * j / DK)
    t = np.arange(T, dtype=np.float64)
    ang = t[None, :] * inv_freq[:, None]          # [32, T]
    return np.cos(ang), np.sin(ang)


def _psw():
    # pure +/-32 swap within each 64-block (signs live in the SQ table)
    M = np.zeros((128, 128), dtype=np.float32)
    for p in range(128):
        M[p, p + 32 if p % 64 < 32 else p - 32] = 1.0
    return np.ascontiguousarray(M.T)


def shard_inputs(x, Wqkv, bqkv, bout, Wout, with_bias=False):
    x = np.asarray(x, dtype=np.float32)
    Wqkv = np.asarray(Wqkv, dtype=np.float32)
    bqkv = np.asarray(bqkv, dtype=np.float32)
    Wout = np.asarray(Wout, dtype=np.float32)

    cos_t, sin_t = _rope_tables()
    cq = np.tile(cos_t, (4, 1)).astype(ml_dtypes.bfloat16)
    # rows 0-31: +sin (consumed by rope dests 32-63); rows 32-63: -sin
    sq = np.tile(np.concatenate([sin_t, -sin_t], axis=0), (2, 1)).astype(ml_dtypes.bfloat16)
    tri = np.triu(np.ones((128, 128), dtype=np.float32)).astype(ml_dtypes.bfloat16)
    psw = _psw().astype(ml_dtypes.bfloat16)

    Wfull = Wqkv

    # XT packed as the SBUF image: [128, (n, k, 512)] where row p, block
    # (n, k) holds x[b][n*512+c, k*128+p]
    xt = {}
    for b in range(B):
        xb = x[b].astype(ml_dtypes.bfloat16)            # [T, D]
        v = xb.reshape(TC512, 512, KC, 128)             # (n, c, k, p)
        xt[b] = np.ascontiguousarray(v.transpose(3, 0, 2, 1).reshape(128, KC * T))

    def pack_kmajor(Wsl):
        # [KC*128, W] -> [128, (k, W)] SBUF image
        Wsl = np.ascontiguousarray(Wsl).astype(ml_dtypes.bfloat16)
        K2, W = Wsl.shape
        return np.ascontiguousarray(
            Wsl.reshape(KC if K2 == KC * 128 else K2 // 128, 128, W)
            .transpose(1, 0, 2).reshape(128, -1))

    in_maps = []
    for c in range(NCORES):
        b = c // 4
        heads = [4 * (c % 4) + i for i in range(HEADS_PER_CORE)]
        # chunk order: [Qp0 | Kp0 | Qp1 | Kp1], each 128 rows (2 heads x 64)
        qk_rows = []
        for p in range(2):
            qrows, krows = [], []
            for h in (2 * p, 2 * p + 1):
                H = heads[h]
                qrows += [H * 192 + j for j in _DEINT]
                krows += [H * 192 + 64 + j for j in _DEINT]
            qk_rows += qrows + krows
        v_rows = []
        for h in range(4):
            H = heads[h]
            v_rows += [H * 192 + 128 + j for j in range(DK)]
        vch_out = []
        for h in range(4):
            H = heads[h]
            vch_out += [H * 64 + j for j in range(DK)]

        im = {
            "XT": xt[b],
            "WQK": pack_kmajor(Wfull[qk_rows].T),
            "WV": pack_kmajor(Wfull[v_rows].T),
            "WOUT": pack_kmajor(Wout[:, vch_out].T),
            "PSW": psw,
            "CQ": cq,
            "SQ": sq,
            "TRI": tri,
        }
        if with_bias:
            im["XLAST"] = np.ones((1, T), ml_dtypes.bfloat16)
            im["WQKLAST"] = np.ascontiguousarray(bqkv[qk_rows][None, :]).astype(ml_dtypes.bfloat16)
            im["WVLAST"] = np.ascontiguousarray(bqkv[v_rows][None, :]).astype(ml_dtypes.bfloat16)
        in_maps.append(im)
    return in_maps


_CACHED = {}


def _get_program(debug=False, with_bias=False):
    key = (bool(debug), bool(with_bias))
    if key not in _CACHED:
        _CACHED[key] = build_program(debug=debug, with_bias=with_bias)
    return _CACHED[key]


def run_cores(inputs, debug=False, trace=False, tmpdir=None):
    with_bias = bool(np.any(np.asarray(inputs["bqkv"], np.float32)))
    nc = _get_program(debug=debug, with_bias=with_bias)
    in_maps = shard_inputs(
        inputs["x"], inputs["Wqkv"], inputs["bqkv"], inputs["bout"],
        inputs["Wout"], with_bias=with_bias)
    res = run_bass_kernel_spmd(
        nc, in_maps, core_ids=list(range(NCORES)), trace=trace, tmpdir=tmpdir,
    )
    return res


def combine(results, bout):
    bout = np.asarray(bout, dtype=np.float32)
    out = np.empty((B, T, D), dtype=np.float32)
    for b in range(B):
        acc = results[4 * b]["OUT"].astype(np.float32)
        for c in range(4 * b + 1, 4 * b + 4):
            acc = acc + results[c]["OUT"].astype(np.float32)
        # unpack [128, (tcc, D)] -> [T, D]
        out[b] = acc.reshape(128, TC128, D).transpose(1, 0, 2).reshape(T, D) + bout[None, :]
    return out


def kernel(x, Wqkv, bqkv, Wout, bout):
    res = run_cores(dict(x=x, Wqkv=Wqkv, bqkv=bqkv, Wout=Wout, bout=bout))
    return combine(res.results, bout)
